# revision 4
# baseline (speedup 1.0000x reference)
"""CSILoss (contrastive + rotation CE) Trainium2 kernel.

Contract: kernel(**inputs) takes the FULL unsharded inputs
  z: [8192, 256] f32, rotation_predictions: [8192, 4] f32, labels: [8192] i64
and returns the full scalar loss (f32), computed on 8 NeuronCores.

Sharding: data-parallel over rows of z. Each core receives the full z (to
build the normalized-transposed embedding matrix znT used as the matmul RHS)
plus its own 1024-row slab (LHS, rotation slab, label one-hots). Each core
computes its 1024x8192 logits slab, the masked logsumexp and positive terms,
and reduces to one scalar partial; the host sums the 8 partials.
"""

import sys

for _p in ("/opt/trn_rl_repo", "/root/.axon_site/_ro/trn_rl_repo"):
    if _p not in sys.path:
        sys.path.insert(0, _p)

import numpy as np

import concourse.bass as bass
import concourse.tile as tile
from concourse import bacc, mybir
from concourse.bass import ds, ts
from concourse.bass_utils import run_bass_kernel_spmd

B, D = 8192, 256
N_CORES = 8
SLAB = B // N_CORES  # 1024 rows per core
RB = SLAB // 128  # 8 row-blocks of 128 per core
TB = B // 128  # 64 total row-blocks
F32 = mybir.dt.float32
BF16 = mybir.dt.bfloat16
AF = mybir.ActivationFunctionType
ALU = mybir.AluOpType

_CACHE = {}


def _build():
    nc = bacc.Bacc("TRN2", target_bir_lowering=False, debug=False)

    z = nc.declare_dram_parameter("z", [B, D], F32, isOutput=False)
    zslab = nc.declare_dram_parameter("zslab", [SLAB, D], F32, isOutput=False)
    rp = nc.declare_dram_parameter("rp", [SLAB, 4], F32, isOutput=False)
    oh = nc.declare_dram_parameter("oh", [SLAB, 4], F32, isOutput=False)
    # [128,128] f32 masks: identity (diag extraction + diag(rnorm) build),
    # pair mask (1 at [p, p^1]).
    idm = nc.declare_dram_parameter("idm", [128, 128], F32, isOutput=False)
    pm = nc.declare_dram_parameter("pm", [128, 128], F32, isOutput=False)
    partial = nc.declare_dram_parameter("partial", [1, 1], F32, isOutput=True)

    with tile.TileContext(nc) as tc:
        from contextlib import ExitStack

        with ExitStack() as stk:
            const = stk.enter_context(tc.tile_pool(name="const", bufs=1))
            small = stk.enter_context(tc.tile_pool(name="small", bufs=1))
            escp = stk.enter_context(tc.tile_pool(name="esc", bufs=2))

            idm_sb = const.tile([128, 128], F32)
            nc.sync.dma_start(out=idm_sb[:], in_=idm[:])
            pm_sb = const.tile([128, 128], F32)
            nc.sync.dma_start(out=pm_sb[:], in_=pm[:])
            rp_sb = const.tile([128, RB, 4], F32)
            nc.sync.dma_start(out=rp_sb[:], in_=rp[:, :].rearrange("(b p) f -> p b f", p=128))
            oh_sb = const.tile([128, RB, 4], F32)
            nc.sync.dma_start(out=oh_sb[:], in_=oh[:, :].rearrange("(b p) f -> p b f", p=128))
            ones = const.tile([128, 1], F32)
            nc.vector.memset(ones[:], 1.0)

            # persistent normalized-transposed embeddings (bf16)
            znT0 = const.tile([128, B], BF16, tag="znT0")  # d in [0,128)
            znT1 = const.tile([128, B], BF16, tag="znT1")  # d in [128,256)
            zsT0 = const.tile([128, SLAB], BF16, tag="zsT0")
            zsT1 = const.tile([128, SLAB], BF16, tag="zsT1")

            sumsq = small.tile([128, TB], F32)
            sumsq_s = small.tile([128, RB], F32)
            posv = small.tile([128, RB], F32)
            diagv = small.tile([128, RB], F32)
            acc = small.tile([128, RB, 4], F32)

            with ExitStack() as p1:
                zin = p1.enter_context(tc.tile_pool(name="zin", bufs=9))
                sqp = p1.enter_context(tc.tile_pool(name="sqp", bufs=2))
                drp = p1.enter_context(tc.tile_pool(name="drp", bufs=8))
                msc = p1.enter_context(tc.tile_pool(name="msc", bufs=2))
                p1ps = p1.enter_context(tc.tile_pool(name="p1ps", bufs=4, space="PSUM"))
                pDps = p1.enter_context(tc.tile_pool(name="pDps", bufs=2, space="PSUM"))

                # ---- load z (8 chunks of 8 row-blocks) + slab, sumsq per row
                z_sb = []
                for g in range(8):
                    t8 = zin.tile([128, 8, D], F32, tag="zc")
                    nc.sync.dma_start(
                        out=t8[:],
                        in_=z[g * 1024 : (g + 1) * 1024, :].rearrange(
                            "(b p) d -> p b d", p=128
                        ),
                    )
                    z_sb.append(t8)
                zs_sb = zin.tile([128, RB, D], F32, tag="zs")
                nc.sync.dma_start(
                    out=zs_sb[:], in_=zslab[:, :].rearrange("(b p) d -> p b d", p=128)
                )

                for t in range(TB):
                    g, b = divmod(t, 8)
                    scr = sqp.tile([128, D], F32, tag="sqscr")
                    nc.vector.scalar_tensor_tensor(
                        out=scr[:],
                        in0=z_sb[g][:, b, :],
                        scalar=1.0,
                        in1=z_sb[g][:, b, :],
                        op0=ALU.mult,
                        op1=ALU.mult,
                        accum_out=sumsq[:, t : t + 1],
                    )
                for b in range(RB):
                    scr = sqp.tile([128, D], F32, tag="sqscr")
                    nc.vector.scalar_tensor_tensor(
                        out=scr[:],
                        in0=zs_sb[:, b, :],
                        scalar=1.0,
                        in1=zs_sb[:, b, :],
                        op0=ALU.mult,
                        op1=ALU.mult,
                        accum_out=sumsq_s[:, b : b + 1],
                    )

                # rnorm = min(exp(-0.5*ln(sumsq)), 1e8)  == 1/max(sqrt(sumsq), 1e-8)
                rnorm = small.tile([128, TB], F32)
                nc.scalar.activation(out=rnorm[:], in_=sumsq[:], func=AF.Ln)
                nc.scalar.activation(out=rnorm[:], in_=rnorm[:], func=AF.Exp, scale=-0.5)
                nc.vector.tensor_scalar_min(out=rnorm[:], in0=rnorm[:], scalar1=1e8)
                rnorm_s = small.tile([128, RB], F32)
                nc.scalar.activation(out=rnorm_s[:], in_=sumsq_s[:], func=AF.Ln)
                nc.scalar.activation(out=rnorm_s[:], in_=rnorm_s[:], func=AF.Exp, scale=-0.5)
                nc.vector.tensor_scalar_min(out=rnorm_s[:], in0=rnorm_s[:], scalar1=1e8)

                # ---- transpose+normalize: znT[:, t*128+j] = z[row j of block t, :]*rnorm
                def transpose_group(tq, n_t, src_of, rn, rn_col, dst0, dst1):
                    # 4 row-blocks -> one [128,512] psum per d-half
                    ps0 = p1ps.tile([128, 512], F32, tag="p1ps")
                    ps1 = p1ps.tile([128, 512], F32, tag="p1ps")
                    for i in range(n_t):
                        t = tq * 4 + i
                        dr_t = drp.tile([128, 128], F32, tag="dr")
                        nc.vector.tensor_scalar_mul(
                            out=dr_t[:], in0=idm_sb[:], scalar1=rn[:, rn_col(t) : rn_col(t) + 1]
                        )
                        src = src_of(t)
                        nc.tensor.matmul(
                            ps0[:, ts(i, 128)], lhsT=src[0], rhs=dr_t[:], start=True, stop=True
                        )
                        nc.tensor.matmul(
                            ps1[:, ts(i, 128)], lhsT=src[1], rhs=dr_t[:], start=True, stop=True
                        )
                    # copy+convert to bf16 (alternate engines)
                    if tq % 2 == 0:
                        nc.vector.tensor_copy(dst0[:, ts(tq, 512)], ps0[:])
                        nc.scalar.copy(dst1[:, ts(tq, 512)], ps1[:])
                    else:
                        nc.scalar.copy(dst0[:, ts(tq, 512)], ps0[:])
                        nc.vector.tensor_copy(dst1[:, ts(tq, 512)], ps1[:])

                for tq in range(16):
                    transpose_group(
                        tq,
                        4,
                        lambda t: (
                            z_sb[t // 8][:, t % 8, 0:128],
                            z_sb[t // 8][:, t % 8, 128:256],
                        ),
                        rnorm,
                        lambda t: t,
                        znT0,
                        znT1,
                    )
                for tq in range(2):
                    transpose_group(
                        tq,
                        4,
                        lambda t: (zs_sb[:, t, 0:128], zs_sb[:, t, 128:256]),
                        rnorm_s,
                        lambda t: t,
                        zsT0,
                        zsT1,
                    )

                # ---- diagonal blocks: pos & diag logits (unscaled cosine sims)
                for rb in range(RB):
                    psD = pDps.tile([128, 128], F32, tag="psD")
                    nc.tensor.matmul(
                        psD[:],
                        lhsT=zsT0[:, ts(rb, 128)],
                        rhs=zsT0[:, ts(rb, 128)],
                        start=True,
                        stop=False,
                    )
                    nc.tensor.matmul(
                        psD[:],
                        lhsT=zsT1[:, ts(rb, 128)],
                        rhs=zsT1[:, ts(rb, 128)],
                        start=False,
                        stop=True,
                    )
                    cpD = msc.tile([128, 128], F32, tag="cpD")
                    nc.vector.tensor_copy(cpD[:], psD[:])
                    mscr = msc.tile([128, 128], F32, tag="mscr")
                    nc.vector.scalar_tensor_tensor(
                        out=mscr[:],
                        in0=cpD[:],
                        scalar=1.0,
                        in1=pm_sb[:],
                        op0=ALU.mult,
                        op1=ALU.mult,
                        accum_out=posv[:, rb : rb + 1],
                    )
                    mscr2 = msc.tile([128, 128], F32, tag="mscr")
                    nc.vector.scalar_tensor_tensor(
                        out=mscr2[:],
                        in0=cpD[:],
                        scalar=1.0,
                        in1=idm_sb[:],
                        op0=ALU.mult,
                        op1=ALU.mult,
                        accum_out=diagv[:, rb : rb + 1],
                    )

            # ---- phase 2: logits slab, exp row-sums
            with ExitStack() as p2:
                p2ps = p2.enter_context(tc.tile_pool(name="p2ps", bufs=2, space="PSUM"))
                for rb in range(RB):
                    for n in range(4):
                        ps = p2ps.tile([128, 2048], F32, tag="p2ps")
                        for s in range(4):
                            j = ds(n * 2048 + s * 512, 512)
                            nc.tensor.matmul(
                                ps[:, ts(s, 512)],
                                lhsT=zsT0[:, ts(rb, 128)],
                                rhs=znT0[:, j],
                                start=True,
                                stop=False,
                            )
                            nc.tensor.matmul(
                                ps[:, ts(s, 512)],
                                lhsT=zsT1[:, ts(rb, 128)],
                                rhs=znT1[:, j],
                                start=False,
                                stop=True,
                            )
                        e = escp.tile([128, 2048], BF16, tag="esc")
                        nc.scalar.activation(
                            out=e[:],
                            in_=ps[:],
                            func=AF.Exp,
                            scale=4.0,
                            accum_out=acc[:, rb, n : n + 1],
                        )

            # ---- phase 3: finals
            S = small.tile([128, RB], F32)
            nc.vector.reduce_sum(S[:], acc[:], axis=mybir.AxisListType.X)
            ed = small.tile([128, RB], F32)
            nc.scalar.activation(out=ed[:], in_=diagv[:], func=AF.Exp, scale=4.0)
            Sm = small.tile([128, RB], F32)
            nc.vector.tensor_tensor(out=Sm[:], in0=S[:], in1=ed[:], op=ALU.subtract)
            lse = small.tile([128, RB], F32)
            nc.scalar.activation(out=lse[:], in_=Sm[:], func=AF.Ln)
            p4 = small.tile([128, RB], F32)
            nc.vector.tensor_scalar_mul(out=p4[:], in0=posv[:], scalar1=4.0)
            lc = small.tile([128, RB], F32)
            nc.vector.tensor_tensor(out=lc[:], in0=lse[:], in1=p4[:], op=ALU.subtract)

            # rotation CE
            rs = small.tile([128, RB], F32)
            rescr = small.tile([128, RB, 4], F32)
            for b in range(RB):
                nc.scalar.activation(
                    out=rescr[:, b, :],
                    in_=rp_sb[:, b, :],
                    func=AF.Exp,
                    accum_out=rs[:, b : b + 1],
                )
            rlse = small.tile([128, RB], F32)
            nc.scalar.activation(out=rlse[:], in_=rs[:], func=AF.Ln)
            picked = small.tile([128, 1], F32)
            pscr = small.tile([128, RB, 4], F32)
            nc.vector.scalar_tensor_tensor(
                out=pscr[:],
                in0=rp_sb[:],
                scalar=1.0,
                in1=oh_sb[:],
                op0=ALU.mult,
                op1=ALU.mult,
                accum_out=picked[:],
            )

            csum = small.tile([128, 1], F32)
            nc.vector.reduce_sum(csum[:], lc[:], axis=mybir.AxisListType.X)
            rsum = small.tile([128, 1], F32)
            nc.vector.reduce_sum(rsum[:], rlse[:], axis=mybir.AxisListType.X)
            tot = small.tile([128, 1], F32)
            nc.vector.tensor_tensor(out=tot[:], in0=csum[:], in1=rsum[:], op=ALU.add)
            nc.vector.tensor_tensor(out=tot[:], in0=tot[:], in1=picked[:], op=ALU.subtract)

            with tc.tile_pool(name="pfin", bufs=1, space="PSUM") as pfin:
                psF = pfin.tile([1, 1], F32)
                nc.tensor.matmul(psF[:], lhsT=tot[:], rhs=ones[:], start=True, stop=True)
                outsb = small.tile([1, 1], F32)
                nc.vector.tensor_copy(outsb[:], psF[:])
                nc.sync.dma_start(out=partial[:], in_=outsb[:])

    nc.compile()
    return nc


def get_nc():
    if "nc" not in _CACHE:
        _CACHE["nc"] = _build()
    return _CACHE["nc"]


def _host_inputs(z, rotation_predictions, labels):
    z = np.ascontiguousarray(np.asarray(z, dtype=np.float32))
    rp = np.ascontiguousarray(np.asarray(rotation_predictions, dtype=np.float32))
    lab = np.asarray(labels).astype(np.int64)
    oh_full = np.eye(4, dtype=np.float32)[lab % 4]  # [B, 4]

    idm = np.eye(128, dtype=np.float32)
    pidx = np.arange(128)
    pmk = np.zeros((128, 128), dtype=np.float32)
    pmk[pidx, pidx ^ 1] = 1.0

    in_maps = []
    for c in range(N_CORES):
        r0, r1 = c * SLAB, (c + 1) * SLAB
        in_maps.append(
            {
                "z": z,
                "zslab": z[r0:r1],
                "rp": rp[r0:r1],
                "oh": oh_full[r0:r1],
                "idm": idm,
                "pm": pmk,
            }
        )
    return in_maps


def kernel(z, rotation_predictions, labels):
    nc = get_nc()
    in_maps = _host_inputs(z, rotation_predictions, labels)
    res = run_bass_kernel_spmd(nc, in_maps, core_ids=list(range(N_CORES)))
    total = sum(float(res.results[c]["partial"][0, 0]) for c in range(N_CORES))
    return np.float32(total / B)


if __name__ == "__main__":
    rng = np.random.default_rng(0)
    z = rng.standard_normal((B, D), dtype=np.float32)
    rp = rng.standard_normal((B, 4), dtype=np.float32)
    lab = rng.integers(0, 4, size=(B,)).astype(np.int64)
    print("loss:", kernel(z, rp, lab))
